# revision 11
# baseline (speedup 1.0000x reference)
"""Multi-head attention (B=2, T=2048, D=1024, H=16, causal) on 8 TRN2 NeuronCores.

Sharding (tensor-parallel heads + token-parallel epilogue):
  - Core c owns heads (2c, 2c+1) -> a 128-wide slice of the QKV output dim.
  - bf16 operands everywhere on the matmul path (f32 PSUM accumulation);
    x / weights are cast to bf16 on the host, halving HBM traffic.
  - Startup DMAs are issue-rate bound (~650ns per dma_start on a queue), so
    the critical path uses few, large dma_starts spread over three issue
    queues (sync: x slabs, scalar: weights, gpsimd: small constants).
  - QKV projections: qT/kT [128, B*T] bf16 feature-major; bias epilogues on
    VectorE (tensor_scalar_add) so ScalarE is reserved for attention exp.
  - Attention: streaming over 128-wide key blocks, transposed score tiles
    S^T [k, q] for both heads in one [128, 1024] PSUM pair; on diagonal
    blocks only the valid suffix [doff, 512) is computed and a 128-wide
    triangular -300 bias tile is accumulated on top (exp underflows to 0);
    one (narrowed) exp per k-block on ScalarE; ctx^T accumulates with an
    appended ones-column in v so row 64 of the accumulator is the softmax
    denominator.  The k-loop is software-pipelined (ctx of block k issues
    after scores of block k+1).
  - Softmax normalize: denominator rows leave PSUM via ScalarE copies, are
    DMA-broadcast across partitions through a DRAM bounce, inverted with
    reciprocal_approx_fast, and applied as wide [64, N] VectorE multiplies.
  - The ctx AllToAll is split into FOUR collectives (2 batches x 2 token
    groups), each triggered as soon as its qblocks finish, so exchanges and
    the token-sharded output projection overlap attention compute; only the
    last quarter remains in the tail.  Host reassembles the token layout.
"""

import numpy as np
import ml_dtypes

import concourse.bacc as bacc
import concourse.bass as bass
import concourse.mybir as mybir
import concourse.tile as tile
from concourse import bass_utils
from concourse.bass import ts

D = 1024
H = 16
DK = D // H  # 64
NCORES = 8
HPC = H // NCORES  # heads per core = 2
DSL = HPC * DK  # per-core QKV output slice = 128
P = 128
QBLK = 512
KBLK = 128
DA = DK + 1  # 65: head dim + ones column (softmax denominator row)
G = 2  # token groups per batch (collective split granularity)

F32 = mybir.dt.float32
F32R = mybir.dt.float32r
BF16 = mybir.dt.bfloat16
EXP = mybir.ActivationFunctionType.Exp

BF16_NP = ml_dtypes.bfloat16


def build_nc(B=2, T=2048):
    """Build the SPMD Bass module (identical program on all 8 cores)."""
    NTOK = B * T
    TPC = NTOK // NCORES  # tokens per core in the output projection
    TPB = T // NCORES  # tokens per core per batch
    TPG = TPB // G  # tokens per core per (batch, group)
    KO = D // P  # 8 contraction chunks
    NKB = T // KBLK  # key blocks per batch
    NQB = T // QBLK  # query blocks per batch
    GQ = NQB // G  # q-blocks per group
    NSLAB = NTOK // QBLK  # x token slabs
    NPAIR = NSLAB // 2
    NDST = QBLK // TPG  # destination cores covered by one q-block
    TW = min(P, TPG)  # outproj token-tile width
    TBG = TPG // TW  # outproj token tiles per (batch, group)

    nc = bacc.Bacc("TRN2", target_bir_lowering=False, debug=False,
                   num_devices=NCORES)

    # ---- DRAM I/O ------------------------------------------------------
    xT_d = nc.dram_tensor("xT", [P, NPAIR, KO, 2 * QBLK], BF16,
                          kind="ExternalInput")
    wqT_d = nc.dram_tensor("wqT", [P, KO, DSL], BF16, kind="ExternalInput")
    wkT_d = nc.dram_tensor("wkT", [P, KO, DSL], BF16, kind="ExternalInput")
    wvT_d = nc.dram_tensor("wvT", [P, KO, DSL], BF16, kind="ExternalInput")
    woT_d = nc.dram_tensor("woT", [P, KO, D], BF16, kind="ExternalInput")
    bq_d = nc.dram_tensor("bq", [DSL, 1], F32, kind="ExternalInput")
    bk_d = nc.dram_tensor("bk", [DSL, 1], F32, kind="ExternalInput")
    bv_d = nc.dram_tensor("bv", [DSL, 1], F32, kind="ExternalInput")
    bo_d = nc.dram_tensor("bo", [D], F32, kind="ExternalInput")
    mask_d = nc.dram_tensor("mask", [P, P], BF16, kind="ExternalInput")
    ident_d = nc.dram_tensor("ident", [P, P], BF16, kind="ExternalInput")
    identr_d = nc.dram_tensor("identr", [P, P], F32R, kind="ExternalInput")
    out_d = nc.dram_tensor("out", [TPC, D], F32, kind="ExternalOutput")

    with tile.TileContext(nc) as tc:
        with (
            tc.tile_pool(name="consts", bufs=1) as consts,
            tc.tile_pool(name="acts", bufs=1) as acts,
            tc.tile_pool(name="xin", bufs=4) as xin,
            tc.tile_pool(name="attn", bufs=3) as attn_pool,
            tc.tile_pool(name="small", bufs=2) as small,
            tc.tile_pool(name="outg", bufs=1) as outg,
            tc.tile_pool(name="outp", bufs=4) as outp,
            tc.tile_pool(name="psA", bufs=2, space="PSUM") as psA,
            tc.tile_pool(name="psC", bufs=2, space="PSUM") as psC,
            tc.tile_pool(name="dram", bufs=2, space="DRAM") as dram,
        ):
            # ---- startup: few large DMAs, three issue queues ------------
            wq_sb = consts.tile([P, KO, DSL], BF16, tag="wq")
            wk_sb = consts.tile([P, KO, DSL], BF16, tag="wk")
            wv_sb = consts.tile([P, KO, DSL], BF16, tag="wv")
            bq_sb = consts.tile([P, 1], F32, tag="bq")
            bk_sb = consts.tile([P, 1], F32, tag="bk")
            bv_sb = consts.tile([P, 1], F32, tag="bv")

            # pair-0 x slab in two chunks so the first matmul starts early
            xt0 = xin.tile([P, KO, 2 * QBLK], BF16, tag="xt", name="xt0")
            nc.sync.dma_start(xt0[:, 0:2], xT_d.ap()[:, 0, 0:2])
            nc.scalar.dma_start(wq_sb[:], wqT_d.ap())
            nc.sync.dma_start(xt0[:, 2:KO], xT_d.ap()[:, 0, 2:KO])
            nc.scalar.dma_start(wk_sb[:], wkT_d.ap())
            nc.scalar.dma_start(wv_sb[:], wvT_d.ap())
            nc.gpsimd.dma_start(bq_sb[:], bq_d.ap())
            nc.gpsimd.dma_start(bk_sb[:], bk_d.ap())
            nc.gpsimd.dma_start(bv_sb[:], bv_d.ap())

            identr_sb = consts.tile([P, P], F32R, tag="identr")
            nc.gpsimd.dma_start(identr_sb[:], identr_d.ap())
            ident_sb = consts.tile([P, P], BF16, tag="ident")
            nc.gpsimd.dma_start(ident_sb[:], ident_d.ap())
            mask_sb = consts.tile([P, P], BF16, tag="mask")
            nc.gpsimd.dma_start(mask_sb[:], mask_d.ap())

            qT = acts.tile([P, NTOK], BF16, tag="qT")
            kT = acts.tile([P, NTOK], BF16, tag="kT")
            vT = acts.tile([P, NTOK], F32R, tag="vT")
            v_nat = acts.tile([P, NTOK // P, 2 * DA], BF16, tag="v_nat")
            nc.gpsimd.memset(v_nat[:, :, DK], 1.0)
            nc.gpsimd.memset(v_nat[:, :, DA + DK], 1.0)

            # tiny dummy collective to absorb the first-collective arming
            # cost (~11us) during the startup phase
            warm_in = dram.tile([NCORES, 2], BF16, tag="warm_in",
                                name="warm_in")
            warm_out = dram.tile([NCORES, 2], BF16, tag="warm_out",
                                 name="warm_out")
            nc.gpsimd.dma_start(warm_in[:], mask_d.ap()[0:NCORES, 0:2])
            nc.gpsimd.collective_compute(
                "AllToAll",
                mybir.AluOpType.bypass,
                replica_groups=[list(range(NCORES))],
                ins=[warm_in[:].opt()],
                outs=[warm_out[:].opt()],
            )

            def proj_pair(i, xt):
                """QKV projections for token slabs 2i, 2i+1.  Bias epilogue
                on VectorE (frees ScalarE for attention exp)."""
                for w_sb, b_sb, dst in ((wq_sb, bq_sb, qT),
                                        (wk_sb, bk_sb, kT),
                                        (wv_sb, bv_sb, vT)):
                    ps = psA.tile([P, 2 * QBLK], F32, tag="sp", name="ps")
                    for ko in range(KO):
                        nc.tensor.matmul(ps[:, 0:QBLK], w_sb[:, ko],
                                         xt[:, ko, 0:QBLK], start=(ko == 0),
                                         stop=(ko == KO - 1))
                        nc.tensor.matmul(ps[:, QBLK:], w_sb[:, ko],
                                         xt[:, ko, QBLK:], start=(ko == 0),
                                         stop=(ko == KO - 1))
                    nc.vector.tensor_scalar_add(dst[:, ts(i, 2 * QBLK)],
                                                ps[:], b_sb[:, 0:1])

            def load_pair(i):
                xt = xin.tile([P, KO, 2 * QBLK], BF16, tag="xt",
                              name=f"xt{i}")
                nc.sync.dma_start(xt[:], xT_d.ap()[:, i])
                return xt

            def v_nat_block(j):
                """Transpose one [128,128] vT tile into v_nat (both heads),
                leaving the ones columns intact."""
                ptf = psA.tile([P, 2 * QBLK], F32R, tag="sp", name="ptf")
                pt = ptf[:, :P]
                nc.tensor.transpose(pt[:], vT[:, ts(j, P)], identr_sb[:])
                nc.vector.tensor_copy(v_nat[:, j, 0:DK], pt[:, 0:DK])
                nc.vector.tensor_copy(v_nat[:, j, DA:DA + DK], pt[:, DK:P])

            a2a_in = [dram.tile([NCORES, P, TPG], BF16, tag=f"a2a_in{b}{g}",
                                name=f"a2a_in{b}{g}")
                      for b in range(B) for g in range(G)]
            a2a_out = [dram.tile([NCORES, P, TPG], BF16, tag=f"a2a_out{b}{g}",
                                 name=f"a2a_out{b}{g}")
                       for b in range(B) for g in range(G)]

            def attention_qblock(b, qi):
                q_sl = ts(b * T // QBLK + qi, QBLK)
                nkb = (qi + 1) * (QBLK // KBLK)
                C0 = psC.tile([P, QBLK], F32, tag="ctx0", name="C0")
                C1 = psC.tile([P, QBLK], F32, tag="ctx1", name="C1")

                def emit_ctx(pend):
                    ap_, jjp, doff, st, sp = pend
                    d = max(doff, 0)
                    nc.tensor.matmul(C0[0:DA, d:], v_nat[:, jjp, 0:DA],
                                     ap_[:, d:QBLK], start=st, stop=sp,
                                     skip_group_check=True)
                    nc.tensor.matmul(C1[0:DA, d:], v_nat[:, jjp, DA:2 * DA],
                                     ap_[:, QBLK + d:], start=st, stop=sp,
                                     skip_group_check=True)

                pend = None
                for ki in range(nkb):
                    k_sl = ts(b * T // KBLK + ki, KBLK)
                    jj = b * NKB + ki
                    doff = ki * KBLK - qi * QBLK
                    diag = doff >= 0
                    d = max(doff, 0)
                    sp_t = psA.tile([P, 2 * QBLK], F32, tag="sp", name="sp_t")
                    nc.tensor.matmul(sp_t[:, d:QBLK],
                                     kT[0:DK, k_sl], qT[0:DK, q_sl][:, d:],
                                     start=True, stop=not diag,
                                     tile_position=(0, 0),
                                     skip_group_check=diag)
                    nc.tensor.matmul(sp_t[:, QBLK + d:],
                                     kT[DK:P, k_sl], qT[DK:P, q_sl][:, d:],
                                     start=True, stop=not diag,
                                     tile_position=(64, 0),
                                     skip_group_check=diag)
                    if diag:
                        # triangular -300 bias over the 128-wide diagonal
                        nc.tensor.matmul(sp_t[:, d:d + KBLK], ident_sb[:],
                                         mask_sb[:], start=False, stop=True,
                                         skip_group_check=True)
                        nc.tensor.matmul(sp_t[:, QBLK + d:QBLK + d + KBLK],
                                         ident_sb[:], mask_sb[:],
                                         start=False, stop=True,
                                         skip_group_check=True)
                    a_p = attn_pool.tile([P, 2 * QBLK], BF16, tag="ap",
                                         name="a_p")
                    sp_v = sp_t[:].rearrange("p (h q) -> p h q", h=2)
                    ap_v = a_p[:].rearrange("p (h q) -> p h q", h=2)
                    nc.scalar.activation(ap_v[:, :, d:], sp_v[:, :, d:], EXP)
                    # software pipeline: ctx of the previous k-block issues
                    # after this block's scores, so PE runs ahead of ACT.
                    if pend is not None:
                        emit_ctx(pend)
                    pend = (a_p, jj, doff, ki == 0, ki == nkb - 1)
                emit_ctx(pend)

                # normalize ctx^T by 1/denominator (row 64): pull the
                # denominator rows out of PSUM on ScalarE, broadcast across
                # partitions via a DRAM bounce, then wide VectorE ops.
                den_sb = small.tile([DA, 2 * QBLK], F32, tag="den_sb")
                nc.scalar.copy(den_sb[DK:DA, 0:QBLK], C0[DK:DA])
                nc.scalar.copy(den_sb[DK:DA, QBLK:], C1[DK:DA])
                den_dr = dram.tile([1, 2 * QBLK], F32, tag="den_dr",
                                   name="den_dr")
                nc.gpsimd.dma_start(den_dr[:], den_sb[DK:DA, :])
                bc_sb = small.tile([DK, 2 * QBLK], F32, tag="bc_sb")
                nc.gpsimd.dma_start(
                    bc_sb[:], den_dr[:].to_broadcast((DK, 2 * QBLK)))
                rec_sb = small.tile([DK, 2 * QBLK], F32, tag="rec_sb")
                nc.vector.reciprocal_approx_fast(rec_sb[:], bc_sb[:])
                ctx0_sb = small.tile([DK, QBLK], BF16, tag="ctx0_sb")
                ctx1_sb = small.tile([DK, QBLK], BF16, tag="ctx1_sb")
                nc.vector.tensor_mul(ctx0_sb[:], C0[0:DK],
                                     rec_sb[:, 0:QBLK])
                nc.vector.tensor_mul(ctx1_sb[:], C1[0:DK],
                                     rec_sb[:, QBLK:])
                bg = b * G + qi // GQ
                for sub in range(NDST):
                    dst = ((qi % GQ) * QBLK) // TPG + sub
                    nc.sync.dma_start(a2a_in[bg][dst, 0:DK],
                                      ctx0_sb[:, ts(sub, TPG)])
                    nc.sync.dma_start(a2a_in[bg][dst, DK:P],
                                      ctx1_sb[:, ts(sub, TPG)])

            def trigger_coll(bg):
                nc.gpsimd.collective_compute(
                    "AllToAll",
                    mybir.AluOpType.bypass,
                    replica_groups=[list(range(NCORES))],
                    ins=[a2a_in[bg][:].opt()],
                    outs=[a2a_out[bg][:].opt()],
                )

            wo_sb = consts.tile([P, KO, D], BF16, tag="wo")
            bo_sb = consts.tile([P, D], F32, tag="bo")
            nc.scalar.dma_start(wo_sb[:], woT_d.ap())
            nc.gpsimd.dma_start(
                bo_sb[:], bo_d.ap()[None, :].to_broadcast((P, D)))

            ctxg = [outg.tile([P, KO, TPG], BF16, tag=f"ctxg{bg}",
                              name=f"ctxg{bg}") for bg in range(B * G)]

            def gather_group(bg):
                nc.sync.dma_start(
                    ctxg[bg][:], a2a_out[bg][:].rearrange("j p t -> p j t"))

            def outproj_tb(bg, tb):
                b, g = bg // G, bg % G
                po = psA.tile([P, 2 * QBLK], F32, tag="sp", name="po")
                for ko in range(KO):
                    nc.tensor.matmul(po[0:TW, 0:QBLK],
                                     ctxg[bg][:, ko, ts(tb, TW)],
                                     wo_sb[:, ko, 0:QBLK],
                                     start=(ko == 0), stop=(ko == KO - 1))
                    nc.tensor.matmul(po[0:TW, QBLK:],
                                     ctxg[bg][:, ko, ts(tb, TW)],
                                     wo_sb[:, ko, QBLK:],
                                     start=(ko == 0), stop=(ko == KO - 1))
                o_sb = outp.tile([TW, D], F32, tag="o_sb", name=f"o{bg}{tb}")
                nc.vector.tensor_add(o_sb[:], po[0:TW, :], bo_sb[:TW])
                row = b * TPB + g * TPG + tb * TW
                return lambda: nc.gpsimd.dma_start(
                    out_d.ap()[row:row + TW, :], o_sb[:])

            # ---- phase plan ---------------------------------------------
            half_pairs = NPAIR // B  # proj pairs per batch
            xts = [xt0] + [load_pair(i) for i in range(1, NPAIR)]
            for i in range(half_pairs):
                proj_pair(i, xts[i])
            for j in range(NTOK // P // B):
                v_nat_block(j)

            late = []
            for i in range(half_pairs, NPAIR):
                late.append(lambda i=i: proj_pair(i, xts[i]))
            for j0 in range(NTOK // P // B, NTOK // P, 4):
                late.append(lambda j0=j0: [v_nat_block(j)
                                           for j in range(j0, j0 + 4)])

            # batch-0 attention, interleaving batch-1 prep into PE gaps;
            # each token group's AllToAll fires as soon as it completes
            for qi in range(NQB):
                attention_qblock(0, qi)
                if qi % GQ == GQ - 1:
                    trigger_coll(qi // GQ)
                nlate = max(1, (len(late) + NQB - 1 - qi) // (NQB - qi))
                for _ in range(min(nlate, len(late))):
                    late.pop(0)()
            while late:
                late.pop(0)()

            # batch-1 attention: only collective triggers interleave (so
            # they fire the moment their data is ready); gathers, output
            # projections and result writes all run in the tail, where the
            # first three quarters' exchanges have already completed
            for qi in range(NQB):
                attention_qblock(1, qi)
                if qi % GQ == GQ - 1:
                    trigger_coll(G + qi // GQ)
            for bg in range(B * G):
                gather_group(bg)
            writes = []
            for bg in range(B * G):
                for tb in range(TBG):
                    writes.append(outproj_tb(bg, tb))
            for w in writes:
                w()

    nc.compile()
    return nc


_NC_CACHE = {}


def _get_nc(B, T):
    key = (B, T)
    if key not in _NC_CACHE:
        _NC_CACHE[key] = build_nc(B, T)
    return _NC_CACHE[key]


def make_in_maps(x, Wq, bq, Wk, bk, Wv, bv, Wo, bo):
    B, T, _ = x.shape
    NTOK = B * T
    NPAIR = NTOK // (2 * QBLK)
    KO = D // P
    x = np.asarray(x, np.float32)
    # [D, NTOK] -> [p, pair, ko, t]: one contiguous DMA descriptor per
    # partition per (pair, ko) chunk.
    xT = x.reshape(NTOK, D).T  # [D, NTOK]
    xT_t = np.ascontiguousarray(
        xT.reshape(KO, P, NPAIR, 2 * QBLK).transpose(1, 2, 0, 3)
    ).astype(BF16_NP)
    woT = np.ascontiguousarray(
        np.asarray(Wo, np.float32).T.reshape(KO, P, D).transpose(1, 0, 2)
    ).astype(BF16_NP)
    bo = np.asarray(bo, np.float32)
    # triangular -300 bias for the 128-wide diagonal block: keep (0) where
    # the local query column c is >= the local key partition p.
    keep = np.arange(P)[None, :] >= np.arange(P)[:, None]
    mask = np.where(keep, 0.0, -300.0).astype(BF16_NP)
    ident = np.eye(P, dtype=np.float32)
    in_maps = []
    for c in range(NCORES):
        sl = slice(DSL * c, DSL * (c + 1))
        in_maps.append({
            "xT": xT_t,
            "wqT": np.ascontiguousarray(
                (np.asarray(Wq, np.float32)[sl].T * 0.125)
                .reshape(KO, P, DSL).transpose(1, 0, 2)).astype(BF16_NP),
            "wkT": np.ascontiguousarray(
                np.asarray(Wk, np.float32)[sl].T
                .reshape(KO, P, DSL).transpose(1, 0, 2)).astype(BF16_NP),
            "wvT": np.ascontiguousarray(
                np.asarray(Wv, np.float32)[sl].T
                .reshape(KO, P, DSL).transpose(1, 0, 2)).astype(BF16_NP),
            "woT": woT,
            "bq": (np.asarray(bq, np.float32)[sl] * 0.125).reshape(DSL, 1),
            "bk": np.asarray(bk, np.float32)[sl].reshape(DSL, 1),
            "bv": np.asarray(bv, np.float32)[sl].reshape(DSL, 1),
            "bo": bo,
            "mask": mask,
            "ident": ident.astype(BF16_NP),
            "identr": ident,
        })
    return in_maps


LAST_RESULTS = None


def assemble_out(per_core, B, T):
    """per_core[c] is [TPC, D] with rows ordered (batch, group, token)."""
    TPB = T // NCORES
    TPG = TPB // G
    out = np.empty((B, T, D), np.float32)
    for c in range(NCORES):
        oc = per_core[c]
        for b in range(B):
            for g in range(G):
                r = b * TPB + g * TPG
                t = g * (T // G) + c * TPG
                out[b, t:t + TPG] = oc[r:r + TPG]
    return out


def kernel(x, Wq, bq, Wk, bk, Wv, bv, Wo, bo, trace=False, trace_cores=None):
    global LAST_RESULTS
    B, T, _ = x.shape
    assert B == 2
    nc = _get_nc(B, T)
    in_maps = make_in_maps(x, Wq, bq, Wk, bk, Wv, bv, Wo, bo)
    kw = {}
    if trace:
        kw = dict(trace=True, trace_cores=trace_cores)
    res = bass_utils.run_bass_kernel_spmd(nc, in_maps,
                                          core_ids=list(range(NCORES)), **kw)
    LAST_RESULTS = res
    return assemble_out([res.results[c]["out"] for c in range(NCORES)], B, T)


# revision 12
# speedup vs baseline: 1.1752x; 1.1752x over previous
"""Multi-head attention (B=2, T=2048, D=1024, H=16, causal) on 8 TRN2 NeuronCores.

Sharding (tensor-parallel heads + token-parallel epilogue):
  - Core c owns heads (2c, 2c+1) -> a 128-wide slice of the QKV output dim.
  - bf16 operands everywhere on the matmul path (f32 PSUM accumulation);
    x / weights are cast to bf16 on the host, halving HBM traffic.
  - Startup DMAs are issue-rate bound (~650ns per dma_start on a queue), so
    the critical path uses few, large dma_starts spread over three issue
    queues (sync: x slabs, scalar: weights, gpsimd: small constants).
  - QKV projections: qT/kT [128, B*T] bf16 feature-major; bias epilogues on
    VectorE (tensor_scalar_add) so ScalarE is reserved for attention exp.
  - Attention: streaming over 128-wide key blocks, transposed score tiles
    S^T [k, q] for both heads in one [128, 1024] PSUM pair; on diagonal
    blocks only the valid suffix [doff, 512) is computed and a 128-wide
    triangular -300 bias tile is accumulated on top (exp underflows to 0);
    one (narrowed) exp per k-block on ScalarE; ctx^T accumulates with an
    appended ones-column in v so row 64 of the accumulator is the softmax
    denominator.  The k-loop is software-pipelined (ctx of block k issues
    after scores of block k+1).
  - Softmax normalize: denominator rows leave PSUM via ScalarE copies, are
    DMA-broadcast across partitions through a DRAM bounce, inverted with
    reciprocal_approx_fast, and applied as wide [64, N] VectorE multiplies.
  - The ctx AllToAll is split into FOUR collectives (2 batches x 2 token
    groups), each triggered as soon as its qblocks finish, so exchanges and
    the token-sharded output projection overlap attention compute; only the
    last quarter remains in the tail.  Host reassembles the token layout.
"""

import numpy as np
import ml_dtypes

import concourse.bacc as bacc
import concourse.bass as bass
import concourse.mybir as mybir
import concourse.tile as tile
from concourse import bass_utils
from concourse.bass import ts

D = 1024
H = 16
DK = D // H  # 64
NCORES = 8
HPC = H // NCORES  # heads per core = 2
DSL = HPC * DK  # per-core QKV output slice = 128
P = 128
QBLK = 512
KBLK = 128
DA = DK + 1  # 65: head dim + ones column (softmax denominator row)
G = 2  # token groups per batch (collective split granularity)

F32 = mybir.dt.float32
F32R = mybir.dt.float32r
BF16 = mybir.dt.bfloat16
EXP = mybir.ActivationFunctionType.Exp

BF16_NP = ml_dtypes.bfloat16


def build_nc(B=2, T=2048):
    """Build the SPMD Bass module (identical program on all 8 cores)."""
    NTOK = B * T
    TPC = NTOK // NCORES  # tokens per core in the output projection
    TPB = T // NCORES  # tokens per core per batch
    TPG = TPB // G  # tokens per core per (batch, group)
    KO = D // P  # 8 contraction chunks
    NKB = T // KBLK  # key blocks per batch
    NQB = T // QBLK  # query blocks per batch
    GQ = NQB // G  # q-blocks per group
    NSLAB = NTOK // QBLK  # x token slabs
    NPAIR = NSLAB // 2
    NDST = QBLK // TPG  # destination cores covered by one q-block
    TW = min(P, TPG)  # outproj token-tile width
    TBG = TPG // TW  # outproj token tiles per (batch, group)

    nc = bacc.Bacc("TRN2", target_bir_lowering=False, debug=False,
                   num_devices=NCORES)

    # ---- DRAM I/O ------------------------------------------------------
    xT_d = nc.dram_tensor("xT", [P, NPAIR, KO, 2 * QBLK], BF16,
                          kind="ExternalInput")
    wqT_d = nc.dram_tensor("wqT", [P, KO, DSL], BF16, kind="ExternalInput")
    wkT_d = nc.dram_tensor("wkT", [P, KO, DSL], BF16, kind="ExternalInput")
    wvT_d = nc.dram_tensor("wvT", [P, KO, DSL], BF16, kind="ExternalInput")
    woT_d = nc.dram_tensor("woT", [P, KO, D], BF16, kind="ExternalInput")
    bq_d = nc.dram_tensor("bq", [DSL, 1], F32, kind="ExternalInput")
    bk_d = nc.dram_tensor("bk", [DSL, 1], F32, kind="ExternalInput")
    bv_d = nc.dram_tensor("bv", [DSL, 1], F32, kind="ExternalInput")
    bo_d = nc.dram_tensor("bo", [D], F32, kind="ExternalInput")
    mask_d = nc.dram_tensor("mask", [P, P], BF16, kind="ExternalInput")
    ident_d = nc.dram_tensor("ident", [P, P], BF16, kind="ExternalInput")
    identr_d = nc.dram_tensor("identr", [P, P], F32R, kind="ExternalInput")
    out_d = nc.dram_tensor("out", [TPC, D], F32, kind="ExternalOutput")

    with tile.TileContext(nc) as tc:
        with (
            tc.tile_pool(name="consts", bufs=1) as consts,
            tc.tile_pool(name="acts", bufs=1) as acts,
            tc.tile_pool(name="xin", bufs=4) as xin,
            tc.tile_pool(name="attn", bufs=3) as attn_pool,
            tc.tile_pool(name="small", bufs=2) as small,
            tc.tile_pool(name="outg", bufs=1) as outg,
            tc.tile_pool(name="outp", bufs=4) as outp,
            tc.tile_pool(name="psA", bufs=2, space="PSUM") as psA,
            tc.tile_pool(name="psC", bufs=2, space="PSUM") as psC,
            tc.tile_pool(name="dram", bufs=2, space="DRAM") as dram,
        ):
            # ---- startup: few large DMAs, three issue queues ------------
            wq_sb = consts.tile([P, KO, DSL], BF16, tag="wq")
            wk_sb = consts.tile([P, KO, DSL], BF16, tag="wk")
            wv_sb = consts.tile([P, KO, DSL], BF16, tag="wv")
            bq_sb = consts.tile([P, 1], F32, tag="bq")
            bk_sb = consts.tile([P, 1], F32, tag="bk")
            bv_sb = consts.tile([P, 1], F32, tag="bv")

            # pair-0 x slab in two chunks so the first matmul starts early
            xt0 = xin.tile([P, KO, 2 * QBLK], BF16, tag="xt", name="xt0")
            nc.sync.dma_start(xt0[:, 0:2], xT_d.ap()[:, 0, 0:2])
            nc.scalar.dma_start(wq_sb[:], wqT_d.ap())
            nc.sync.dma_start(xt0[:, 2:KO], xT_d.ap()[:, 0, 2:KO])
            nc.scalar.dma_start(wk_sb[:], wkT_d.ap())
            nc.scalar.dma_start(wv_sb[:], wvT_d.ap())
            nc.gpsimd.dma_start(bq_sb[:], bq_d.ap())
            nc.gpsimd.dma_start(bk_sb[:], bk_d.ap())
            nc.gpsimd.dma_start(bv_sb[:], bv_d.ap())

            identr_sb = consts.tile([P, P], F32R, tag="identr")
            nc.gpsimd.dma_start(identr_sb[:], identr_d.ap())
            ident_sb = consts.tile([P, P], BF16, tag="ident")
            nc.gpsimd.dma_start(ident_sb[:], ident_d.ap())
            mask_sb = consts.tile([P, P], BF16, tag="mask")
            nc.gpsimd.dma_start(mask_sb[:], mask_d.ap())

            qT = acts.tile([P, NTOK], BF16, tag="qT")
            kT = acts.tile([P, NTOK], BF16, tag="kT")
            vT = acts.tile([P, NTOK], F32R, tag="vT")
            v_nat = acts.tile([P, NTOK // P, 2 * DA], BF16, tag="v_nat")
            nc.gpsimd.memset(v_nat[:, :, DK], 1.0)
            nc.gpsimd.memset(v_nat[:, :, DA + DK], 1.0)

            # tiny dummy collective to absorb the first-collective arming
            # cost (~11us) during the startup phase
            warm_in = dram.tile([NCORES, 2], BF16, tag="warm_in",
                                name="warm_in")
            warm_out = dram.tile([NCORES, 2], BF16, tag="warm_out",
                                 name="warm_out")
            nc.gpsimd.dma_start(warm_in[:], mask_d.ap()[0:NCORES, 0:2])
            nc.gpsimd.collective_compute(
                "AllToAll",
                mybir.AluOpType.bypass,
                replica_groups=[list(range(NCORES))],
                ins=[warm_in[:].opt()],
                outs=[warm_out[:].opt()],
            )

            def proj_pair(i, xt):
                """QKV projections for token slabs 2i, 2i+1.  Bias epilogue
                on VectorE (frees ScalarE for attention exp)."""
                for w_sb, b_sb, dst in ((wq_sb, bq_sb, qT),
                                        (wk_sb, bk_sb, kT),
                                        (wv_sb, bv_sb, vT)):
                    ps = psA.tile([P, 2 * QBLK], F32, tag="sp", name="ps")
                    for ko in range(KO):
                        nc.tensor.matmul(ps[:, 0:QBLK], w_sb[:, ko],
                                         xt[:, ko, 0:QBLK], start=(ko == 0),
                                         stop=(ko == KO - 1))
                        nc.tensor.matmul(ps[:, QBLK:], w_sb[:, ko],
                                         xt[:, ko, QBLK:], start=(ko == 0),
                                         stop=(ko == KO - 1))
                    nc.vector.tensor_scalar_add(dst[:, ts(i, 2 * QBLK)],
                                                ps[:], b_sb[:, 0:1])

            def load_pair(i):
                xt = xin.tile([P, KO, 2 * QBLK], BF16, tag="xt",
                              name=f"xt{i}")
                nc.sync.dma_start(xt[:], xT_d.ap()[:, i])
                return xt

            def v_nat_block(j):
                """Transpose one [128,128] vT tile into v_nat (both heads),
                leaving the ones columns intact."""
                ptf = psA.tile([P, 2 * QBLK], F32R, tag="sp", name="ptf")
                pt = ptf[:, :P]
                nc.tensor.transpose(pt[:], vT[:, ts(j, P)], identr_sb[:])
                nc.vector.tensor_copy(v_nat[:, j, 0:DK], pt[:, 0:DK])
                nc.vector.tensor_copy(v_nat[:, j, DA:DA + DK], pt[:, DK:P])

            a2a_in = [dram.tile([NCORES, P, TPG], BF16, tag=f"a2a_in{b}{g}",
                                name=f"a2a_in{b}{g}")
                      for b in range(B) for g in range(G)]
            a2a_out = [dram.tile([NCORES, P, TPG], BF16, tag=f"a2a_out{b}{g}",
                                 name=f"a2a_out{b}{g}")
                       for b in range(B) for g in range(G)]

            def attention_qblock(b, qi):
                q_sl = ts(b * T // QBLK + qi, QBLK)
                nkb = (qi + 1) * (QBLK // KBLK)
                C0 = psC.tile([P, QBLK], F32, tag="ctx0", name="C0")
                C1 = psC.tile([P, QBLK], F32, tag="ctx1", name="C1")

                def emit_ctx(pend):
                    ap_, jjp, doff, st, sp = pend
                    d = max(doff, 0)
                    nc.tensor.matmul(C0[0:DA, d:], v_nat[:, jjp, 0:DA],
                                     ap_[:, d:QBLK], start=st, stop=sp,
                                     skip_group_check=True)
                    nc.tensor.matmul(C1[0:DA, d:], v_nat[:, jjp, DA:2 * DA],
                                     ap_[:, QBLK + d:], start=st, stop=sp,
                                     skip_group_check=True)

                pend = None
                for ki in range(nkb):
                    k_sl = ts(b * T // KBLK + ki, KBLK)
                    jj = b * NKB + ki
                    doff = ki * KBLK - qi * QBLK
                    diag = doff >= 0
                    d = max(doff, 0)
                    sp_t = psA.tile([P, 2 * QBLK], F32, tag="sp", name="sp_t")
                    nc.tensor.matmul(sp_t[:, d:QBLK],
                                     kT[0:DK, k_sl], qT[0:DK, q_sl][:, d:],
                                     start=True, stop=not diag,
                                     tile_position=(0, 0),
                                     skip_group_check=diag)
                    nc.tensor.matmul(sp_t[:, QBLK + d:],
                                     kT[DK:P, k_sl], qT[DK:P, q_sl][:, d:],
                                     start=True, stop=not diag,
                                     tile_position=(64, 0),
                                     skip_group_check=diag)
                    if diag:
                        # triangular -300 bias over the 128-wide diagonal
                        nc.tensor.matmul(sp_t[:, d:d + KBLK], ident_sb[:],
                                         mask_sb[:], start=False, stop=True,
                                         skip_group_check=True)
                        nc.tensor.matmul(sp_t[:, QBLK + d:QBLK + d + KBLK],
                                         ident_sb[:], mask_sb[:],
                                         start=False, stop=True,
                                         skip_group_check=True)
                    a_p = attn_pool.tile([P, 2 * QBLK], BF16, tag="ap",
                                         name="a_p")
                    sp_v = sp_t[:].rearrange("p (h q) -> p h q", h=2)
                    ap_v = a_p[:].rearrange("p (h q) -> p h q", h=2)
                    nc.scalar.activation(ap_v[:, :, d:], sp_v[:, :, d:], EXP)
                    # software pipeline: ctx of the previous k-block issues
                    # after this block's scores, so PE runs ahead of ACT.
                    if pend is not None:
                        emit_ctx(pend)
                    pend = (a_p, jj, doff, ki == 0, ki == nkb - 1)
                emit_ctx(pend)

                # normalize ctx^T by 1/denominator (row 64): pull the
                # denominator rows out of PSUM on ScalarE, broadcast across
                # partitions via a DRAM bounce, then wide VectorE ops.
                den_sb = small.tile([DA, 2 * QBLK], F32, tag="den_sb")
                nc.scalar.copy(den_sb[DK:DA, 0:QBLK], C0[DK:DA])
                nc.scalar.copy(den_sb[DK:DA, QBLK:], C1[DK:DA])
                den_dr = dram.tile([1, 2 * QBLK], F32, tag="den_dr",
                                   name="den_dr")
                nc.gpsimd.dma_start(den_dr[:], den_sb[DK:DA, :])
                bc_sb = small.tile([DK, 2 * QBLK], F32, tag="bc_sb")
                nc.gpsimd.dma_start(
                    bc_sb[:], den_dr[:].to_broadcast((DK, 2 * QBLK)))
                rec_sb = small.tile([DK, 2 * QBLK], F32, tag="rec_sb")
                nc.vector.reciprocal_approx_fast(rec_sb[:], bc_sb[:])
                ctx0_sb = small.tile([DK, QBLK], BF16, tag="ctx0_sb")
                ctx1_sb = small.tile([DK, QBLK], BF16, tag="ctx1_sb")
                nc.vector.tensor_mul(ctx0_sb[:], C0[0:DK],
                                     rec_sb[:, 0:QBLK])
                nc.vector.tensor_mul(ctx1_sb[:], C1[0:DK],
                                     rec_sb[:, QBLK:])
                bg = b * G + qi // GQ
                for sub in range(NDST):
                    dst = ((qi % GQ) * QBLK) // TPG + sub
                    nc.sync.dma_start(a2a_in[bg][dst, 0:DK],
                                      ctx0_sb[:, ts(sub, TPG)])
                    nc.sync.dma_start(a2a_in[bg][dst, DK:P],
                                      ctx1_sb[:, ts(sub, TPG)])

            def trigger_coll(bg):
                nc.gpsimd.collective_compute(
                    "AllToAll",
                    mybir.AluOpType.bypass,
                    replica_groups=[list(range(NCORES))],
                    ins=[a2a_in[bg][:].opt()],
                    outs=[a2a_out[bg][:].opt()],
                )

            wo_sb = consts.tile([P, KO, D], BF16, tag="wo")
            bo_sb = consts.tile([P, D], F32, tag="bo")

            def load_wo():
                nc.scalar.dma_start(wo_sb[:], woT_d.ap())
                nc.gpsimd.dma_start(
                    bo_sb[:], bo_d.ap()[None, :].to_broadcast((P, D)))

            ctxg = [outg.tile([P, KO, TPG], BF16, tag=f"ctxg{bg}",
                              name=f"ctxg{bg}") for bg in range(B * G)]

            def gather_group(bg):
                nc.sync.dma_start(
                    ctxg[bg][:], a2a_out[bg][:].rearrange("j p t -> p j t"))

            def outproj_tb(bg, tb):
                b, g = bg // G, bg % G
                po = psA.tile([P, 2 * QBLK], F32, tag="sp", name="po")
                for ko in range(KO):
                    nc.tensor.matmul(po[0:TW, 0:QBLK],
                                     ctxg[bg][:, ko, ts(tb, TW)],
                                     wo_sb[:, ko, 0:QBLK],
                                     start=(ko == 0), stop=(ko == KO - 1))
                    nc.tensor.matmul(po[0:TW, QBLK:],
                                     ctxg[bg][:, ko, ts(tb, TW)],
                                     wo_sb[:, ko, QBLK:],
                                     start=(ko == 0), stop=(ko == KO - 1))
                o_sb = outp.tile([TW, D], F32, tag="o_sb", name=f"o{bg}{tb}")
                nc.vector.tensor_add(o_sb[:], po[0:TW, :], bo_sb[:TW])
                row = b * TPB + g * TPG + tb * TW
                return lambda: nc.gpsimd.dma_start(
                    out_d.ap()[row:row + TW, :], o_sb[:])

            # ---- phase plan ---------------------------------------------
            half_pairs = NPAIR // B  # proj pairs per batch
            xts = [xt0] + [load_pair(i) for i in range(1, half_pairs)]
            for i in range(half_pairs):
                proj_pair(i, xts[i])
            for j in range(NTOK // P // B):
                v_nat_block(j)

            # batch-1 x slabs + output-projection weights stream in during
            # early batch-0 attention -- after the startup burst, before the
            # first collective needs the wire
            late = [lambda: xts.extend(load_pair(i)
                                       for i in range(half_pairs, NPAIR)),
                    load_wo]
            for i in range(half_pairs, NPAIR):
                late.append(lambda i=i: proj_pair(i, xts[i]))
            for j0 in range(NTOK // P // B, NTOK // P, 4):
                late.append(lambda j0=j0: [v_nat_block(j)
                                           for j in range(j0, j0 + 4)])

            # batch-0 attention, interleaving batch-1 prep into PE gaps;
            # each token group's AllToAll fires as soon as it completes
            for qi in range(NQB):
                attention_qblock(0, qi)
                if qi % GQ == GQ - 1:
                    trigger_coll(qi // GQ)
                nlate = max(1, (len(late) + NQB - 1 - qi) // (NQB - qi))
                for _ in range(min(nlate, len(late))):
                    late.pop(0)()
            while late:
                late.pop(0)()

            # batch-1 attention: only collective triggers interleave (so
            # they fire the moment their data is ready); gathers, output
            # projections and result writes all run in the tail, where the
            # first three quarters' exchanges have already completed
            for qi in range(NQB):
                attention_qblock(1, qi)
                if qi % GQ == GQ - 1:
                    trigger_coll(G + qi // GQ)
            for bg in range(B * G):
                gather_group(bg)
            writes = []
            for bg in range(B * G):
                for tb in range(TBG):
                    writes.append(outproj_tb(bg, tb))
            for w in writes:
                w()

    nc.compile()
    return nc


_NC_CACHE = {}


def _get_nc(B, T):
    key = (B, T)
    if key not in _NC_CACHE:
        _NC_CACHE[key] = build_nc(B, T)
    return _NC_CACHE[key]


def make_in_maps(x, Wq, bq, Wk, bk, Wv, bv, Wo, bo):
    B, T, _ = x.shape
    NTOK = B * T
    NPAIR = NTOK // (2 * QBLK)
    KO = D // P
    x = np.asarray(x, np.float32)
    # [D, NTOK] -> [p, pair, ko, t]: one contiguous DMA descriptor per
    # partition per (pair, ko) chunk.
    xT = x.reshape(NTOK, D).T  # [D, NTOK]
    xT_t = np.ascontiguousarray(
        xT.reshape(KO, P, NPAIR, 2 * QBLK).transpose(1, 2, 0, 3)
    ).astype(BF16_NP)
    woT = np.ascontiguousarray(
        np.asarray(Wo, np.float32).T.reshape(KO, P, D).transpose(1, 0, 2)
    ).astype(BF16_NP)
    bo = np.asarray(bo, np.float32)
    # triangular -300 bias for the 128-wide diagonal block: keep (0) where
    # the local query column c is >= the local key partition p.
    keep = np.arange(P)[None, :] >= np.arange(P)[:, None]
    mask = np.where(keep, 0.0, -300.0).astype(BF16_NP)
    ident = np.eye(P, dtype=np.float32)
    in_maps = []
    for c in range(NCORES):
        sl = slice(DSL * c, DSL * (c + 1))
        in_maps.append({
            "xT": xT_t,
            "wqT": np.ascontiguousarray(
                (np.asarray(Wq, np.float32)[sl].T * 0.125)
                .reshape(KO, P, DSL).transpose(1, 0, 2)).astype(BF16_NP),
            "wkT": np.ascontiguousarray(
                np.asarray(Wk, np.float32)[sl].T
                .reshape(KO, P, DSL).transpose(1, 0, 2)).astype(BF16_NP),
            "wvT": np.ascontiguousarray(
                np.asarray(Wv, np.float32)[sl].T
                .reshape(KO, P, DSL).transpose(1, 0, 2)).astype(BF16_NP),
            "woT": woT,
            "bq": (np.asarray(bq, np.float32)[sl] * 0.125).reshape(DSL, 1),
            "bk": np.asarray(bk, np.float32)[sl].reshape(DSL, 1),
            "bv": np.asarray(bv, np.float32)[sl].reshape(DSL, 1),
            "bo": bo,
            "mask": mask,
            "ident": ident.astype(BF16_NP),
            "identr": ident,
        })
    return in_maps


LAST_RESULTS = None


def assemble_out(per_core, B, T):
    """per_core[c] is [TPC, D] with rows ordered (batch, group, token)."""
    TPB = T // NCORES
    TPG = TPB // G
    out = np.empty((B, T, D), np.float32)
    for c in range(NCORES):
        oc = per_core[c]
        for b in range(B):
            for g in range(G):
                r = b * TPB + g * TPG
                t = g * (T // G) + c * TPG
                out[b, t:t + TPG] = oc[r:r + TPG]
    return out


def kernel(x, Wq, bq, Wk, bk, Wv, bv, Wo, bo, trace=False, trace_cores=None):
    global LAST_RESULTS
    B, T, _ = x.shape
    assert B == 2
    nc = _get_nc(B, T)
    in_maps = make_in_maps(x, Wq, bq, Wk, bk, Wv, bv, Wo, bo)
    kw = {}
    if trace:
        kw = dict(trace=True, trace_cores=trace_cores)
    res = bass_utils.run_bass_kernel_spmd(nc, in_maps,
                                          core_ids=list(range(NCORES)), **kw)
    LAST_RESULTS = res
    return assemble_out([res.results[c]["out"] for c in range(NCORES)], B, T)
